# revision 1
# baseline (speedup 1.0000x reference)
"""Braid causal self-attention Trainium2 kernel (8-core SPMD).

Sharding: data-parallel over batch (2) x tensor-parallel over head groups (4).
Core c handles batch b=c//4, q-heads [4g:4g+4], kv-heads [2g:2g+2], g=c%4.
Each core computes a partial projection output (Wproj input-dim shard);
partials are summed on the host.

Key algebraic restructurings (validated vs reference in fp64):
  - q/k are only needed through the braid scores s_q/s_k. With
    g[d,t] = braid/rotary-folded weights and msq[d,t] = cos^2+sin^2,
    s = (sum_d q[d,t]*g[d,t]) * rsqrt(mean_d q[d,t]^2*msq[d,t] + eps),
    so rotary+rmsnorm are never materialized.
  - attn = sigmoid(s_k[j] + s_q[i]) is generated per 128-key block as a
    K=2 matmul from [2,N] slices of a score vector tile (value row + ones
    row), evaluated by the scalar engine's sigmoid, causally masked with a
    triangular multiply, and contracted with v via fp32r matmuls.
"""
import numpy as np
from contextlib import ExitStack

import concourse.bass as bass
import concourse.mybir as mybir
import concourse.tile as tile
from concourse import bacc
from concourse.bass_utils import run_bass_kernel_spmd

F32 = mybir.dt.float32
F32R = mybir.dt.float32r
AF = mybir.ActivationFunctionType

T = 2048
C = 1024
D = 64
EPS = 1e-6
NCORES = 8


def build_program():
    nc = bacc.Bacc()
    dp = nc.declare_dram_parameter
    xT_d = dp("xT", [C, T], F32, isOutput=False)          # x[b].T
    wq_d = dp("wq", [C, 256], F32, isOutput=False)        # Wq[group].T
    wk_d = dp("wk", [C, 128], F32, isOutput=False)
    wv_d = dp("wv", [C, 128], F32, isOutput=False)
    wp_d = dp("wp", [256, C], F32, isOutput=False)        # Wproj[:, group].T (prescaled)
    gm_d = dp("gm", [128, T], F32, isOutput=False)        # braid g (2-head dup)
    mh_d = dp("mh", [128, T], F32, isOutput=False)        # sqrt(cos^2+sin^2) (2-head dup)
    sel_d = dp("sel", [128, 3, 6], F32, isOutput=False)   # head selector masks
    trz_d = dp("trz", [128, 512], F32, isOutput=False)    # [zeros(384) | tri(i>=j)]
    idn_d = dp("idn", [128, 128], F32, isOutput=False)
    ones_d = dp("ones", [1, T], F32, isOutput=False)
    onz_d = dp("onz", [2, 128], F32, isOutput=False)      # [zeros; ones] K=2 bcast pair
    out_d = dp("outp", [T, C], F32, isOutput=True)
    out1_d = dp("outp1", [T, C], F32, isOutput=True)

    with tile.TileContext(nc) as tc, \
         nc.allow_low_precision("float32r output tags are bit-identical fp32"), \
         ExitStack() as ctx:
        cons = ctx.enter_context(tc.tile_pool(name="cons", bufs=1))
        work = ctx.enter_context(tc.tile_pool(name="work", bufs=1))

        # ---- constants / weights in SBUF ----
        wq_s = cons.tile([128, 8, 256], F32R)
        wk_s = cons.tile([128, 8, 128], F32R)
        wv_s = cons.tile([128, 8, 128], F32R)
        wp_s = cons.tile([128, 2, C], F32R)
        sel_s = cons.tile([128, 3, 6], F32R)
        trz_s = cons.tile([128, 512], F32R)
        idn_s = cons.tile([128, 128], F32)
        eps_t = cons.tile([128, 1], F32)
        nc.sync.dma_start(out=wq_s[:], in_=wq_d.ap().rearrange("(kt p) m -> p kt m", p=128).bitcast(F32R))
        nc.sync.dma_start(out=wk_s[:], in_=wk_d.ap().rearrange("(kt p) m -> p kt m", p=128).bitcast(F32R))
        nc.sync.dma_start(out=wv_s[:], in_=wv_d.ap().rearrange("(kt p) m -> p kt m", p=128).bitcast(F32R))
        nc.sync.dma_start(out=wp_s[:], in_=wp_d.ap().rearrange("(kt p) m -> p kt m", p=128).bitcast(F32R))
        nc.sync.dma_start(out=sel_s[:], in_=sel_d.ap().bitcast(F32R))
        nc.sync.dma_start(out=trz_s[:], in_=trz_d.ap().bitcast(F32R))
        nc.sync.dma_start(out=idn_s[:], in_=idn_d.ap())
        nc.vector.memset(eps_t[:], EPS)

        # long-lived work tiles
        vT = work.tile([128, T], F32)
        v_td = work.tile([128, T], F32R)  # 16 blocks of [t128, oc128]
        scomp = work.tile([6, T], F32)
        stil = work.tile([6, T], F32)     # s-tilde accumulator (pre-rsqrt)
        kcolT = work.tile([128, 2, 16], F32)   # s_k columns: [j, kh, jb]
        r1 = work.tile([6, T], F32)
        rq = work.tile([6, T], F32)
        yt0 = work.tile([128, T], F32R)  # heads 0,1 output (d-major)
        yt1 = work.tile([128, T], F32R)

        # ==== phase 1: projections with fused braid reductions ====
        # cn-major: for each 512-column chunk, project all four row-tiles
        # (q0, q1, k, v), then immediately compute the braid products from
        # PSUM and fold them into the per-chunk selector matmuls.
        with tc.tile_pool(name="bpool", bufs=2) as bp, \
             tc.tile_pool(name="xpool", bufs=1) as xp, \
             tc.tile_pool(name="pp1", bufs=2, space="PSUM") as pp1, \
             tc.tile_pool(name="pp2", bufs=2, space="PSUM") as pp2:
            gm_s = bp.tile([128, T], F32, tag="gm")
            mh_s = bp.tile([128, T], F32, tag="mh")
            nc.sync.dma_start(out=gm_s[:], in_=gm_d.ap())
            nc.sync.dma_start(out=mh_s[:], in_=mh_d.ap())
            xT_s = xp.tile([128, 8, T], F32R)
            nc.sync.dma_start(out=xT_s[:],
                              in_=xT_d.ap().rearrange("(kt p) t -> p kt t", p=128).bitcast(F32R))

            tiles = [(wq_s, 0, 0), (wq_s, 128, 1), (wk_s, 0, 2), (wv_s, 0, 3)]
            for cn in range(4):
                sl = slice(512 * cn, 512 * cn + 512)
                pss_t = pp2.tile([6, 512], F32, tag="pss")
                psq_t = pp2.tile([6, 512], F32, tag="psq")
                for w_s, oc0, t_i in tiles:
                    ps = pp1.tile([128, 512], F32, tag="pj")
                    for kt in range(8):
                        nc.tensor.matmul(
                            ps[:], w_s[:, kt, oc0:oc0 + 128],
                            xT_s[:, kt, sl],
                            start=(kt == 0), stop=(kt == 7))
                    if t_i == 3:
                        nc.vector.tensor_copy(vT[:, sl], ps[:])
                    else:
                        a_t = bp.tile([128, 512], F32R, tag="a")
                        b_t = bp.tile([128, 512], F32, tag="b")
                        b2_t = bp.tile([128, 512], F32R, tag="b2")
                        nc.vector.tensor_mul(a_t[:], ps[:], gm_s[:, sl])
                        nc.vector.tensor_mul(b_t[:], ps[:], mh_s[:, sl])
                        nc.vector.tensor_mul(b2_t[:], b_t[:], b_t[:])
                        nc.tensor.matmul(pss_t[:], sel_s[:, t_i, :], a_t[:],
                                         start=(t_i == 0), stop=(t_i == 2))
                        nc.tensor.matmul(psq_t[:], sel_s[:, t_i, :], b2_t[:],
                                         start=(t_i == 0), stop=(t_i == 2))
                # stash s-tilde and ln(ss/64 + eps) for this chunk
                nc.vector.tensor_copy(stil[:, sl], pss_t[:])
                nc.scalar.activation(r1[:, sl], psq_t[:], AF.Ln,
                                     bias=eps_t[0:6], scale=1.0 / 64.0)

            # v transposes: 16 x [128,128] -> v_td blocks
            for grp in range(4):
                ps_t = pp1.tile([128, 512], F32, tag="vtp")
                for k in range(4):
                    jb = 4 * grp + k
                    nc.tensor.transpose(
                        ps_t[:, 128 * k:128 * k + 128],
                        vT[:, 128 * jb:128 * jb + 128], idn_s[:])
                nc.vector.tensor_copy(v_td[:, 512 * grp:512 * grp + 512], ps_t[:])

        # rsqrt via exp(-0.5*ln(.)) and final braid scores
        nc.scalar.activation(rq[:], r1[:], AF.Exp, scale=-0.5)
        nc.vector.tensor_mul(scomp[:], stil[:], rq[:])

        # kcolT: s_k columns via a DRAM bounce (free transpose in the APs)
        ksc_d = nc.dram_tensor("kscratch", [2, T], F32)
        nc.sync.dma_start(out=ksc_d.ap(), in_=scomp[0:2, :])
        nc.sync.dma_start(
            out=kcolT[:],
            in_=ksc_d.ap().rearrange("r (b j) -> j r b", j=128))

        # ================= phase 3: attention =================
        with tc.tile_pool(name="svpool", bufs=1) as svp, \
             tc.tile_pool(name="atpool", bufs=6) as atp, \
             tc.tile_pool(name="pp3", bufs=2, space="PSUM") as pp3:
            # sv: blocks 0-1 = {sk_h, ones}; blocks 2-5 = {ones, sq_h};
            # zero-pair block at [6T, 6T+128) for the s_q broadcast matmul
            sv = svp.tile([2, 6 * T + 128], F32R)
            nc.sync.dma_start(out=sv[0:1, 0:2 * T], in_=scomp[0:2, :].bitcast(F32R))
            nc.sync.dma_start(out=sv[1:2, 0:2 * T],
                              in_=ones_d.ap().to_broadcast((2, T)).bitcast(F32R))
            nc.sync.dma_start(out=sv[0:1, 2 * T:6 * T],
                              in_=ones_d.ap().to_broadcast((4, T)).bitcast(F32R))
            nc.sync.dma_start(out=sv[1:2, 2 * T:6 * T], in_=scomp[2:6, :].bitcast(F32R))
            nc.sync.dma_start(out=sv[0:2, 6 * T:6 * T + 128], in_=onz_d.ap().bitcast(F32R))

            for h in range(4):
                kh = h // 2
                qbase = 2 * T + T * h
                for hs in (0, 1024):
                    y_ps = pp3.tile([64, 1024], F32, tag="yps")
                    # broadcast s_q over all partitions once per (h, half):
                    # out[j,i] = 0*1 + 1*s_q[i] via the [zeros; ones] pair
                    sqb = pp3.tile([128, 1024], F32, tag="sqb")
                    for off in (0, 512):
                        nc.tensor.matmul(
                            sqb[:, off:off + 512],
                            sv[0:2, 6 * T:6 * T + 128],
                            sv[0:2, qbase + hs + off:qbase + hs + off + 512],
                            start=True, stop=True)
                    jmax = (hs + 1024) // 128
                    # 512-aligned windows (f32r matmuls write full 512-wide
                    # psum bank windows; accumulation requires alignment)
                    first_w = [None] * 2
                    last_w = [None] * 2
                    spans = {}
                    for jb in range(jmax):
                        ws = (max(hs, 128 * jb) // 512) * 512
                        spans[jb] = ws
                        for ck in range((ws - hs) // 512, 2):
                            if first_w[ck] is None:
                                first_w[ck] = jb
                            last_w[ck] = jb
                    for jb in range(jmax):
                        ws = spans[jb]
                        W = hs + 1024 - ws
                        vstart = max(hs, 128 * jb)
                        at_t = atp.tile([128, 1024], F32R, tag="att")
                        # attn = sigmoid(s_q[i] + s_k[j]): s_k column as ACT bias.
                        # Only the causal width is computed; the [ws, vstart)
                        # strip holds stale-but-finite data that the mask zeroes.
                        nc.scalar.activation(at_t[:, vstart - ws:W],
                                             sqb[:, vstart - hs:1024],
                                             AF.Sigmoid, bias=kcolT[:, kh, jb:jb + 1])
                        # causal mask: zero the sub-diagonal strip and apply the
                        # triangular mask on the diagonal block in one multiply
                        # against [zeros(384) | tri]
                        strip = 128 * jb - ws
                        mw = strip + (128 if 128 * jb >= hs else 0)
                        if mw > 0:
                            nc.vector.tensor_mul(at_t[:, 0:mw], at_t[:, 0:mw],
                                                 trz_s[:, 384 - strip:384 - strip + mw])
                        # attn @ v accumulation (512-wide, bank-aligned)
                        for off in range(0, W, 512):
                            ck = (ws - hs + off) // 512
                            nc.tensor.matmul(
                                y_ps[:, ws - hs + off:ws - hs + off + 512],
                                v_td[:, 128 * jb + 64 * kh:128 * jb + 64 * kh + 64],
                                at_t[:, off:off + 512],
                                start=(first_w[ck] == jb), stop=(last_w[ck] == jb))
                    yt_dst = yt0 if h < 2 else yt1
                    r0 = 64 * (h % 2)
                    nc.vector.tensor_copy(yt_dst[r0:r0 + 64, hs:hs + 1024], y_ps[:])

        # ================= phase 4: output projection =================
        # split by K-half: the yt0 half is emitted right after heads 0-1
        # finish, overlapping with heads 2-3 attention; halves summed on host
        with tc.tile_pool(name="ostage", bufs=4) as osp, \
             tc.tile_pool(name="pp4", bufs=4, space="PSUM") as pp4:
            for k2, (yt_src, od) in enumerate([(yt0, out_d), (yt1, out1_d)]):
                for tt in range(16):
                    for cn in range(2):
                        ps_o = pp4.tile([128, 512], F32, tag="opj")
                        nc.tensor.matmul(ps_o[:],
                                         yt_src[:, 128 * tt:128 * tt + 128],
                                         wp_s[:, k2, 512 * cn:512 * cn + 512],
                                         start=True, stop=True)
                        o_t = osp.tile([128, 512], F32, tag="ost")
                        nc.vector.tensor_copy(o_t[:], ps_o[:])
                        nc.sync.dma_start(
                            out=od.ap()[128 * tt:128 * tt + 128, 512 * cn:512 * cn + 512],
                            in_=o_t[:])

    nc.compile()
    return nc


_PROGRAM = None


def _get_program():
    global _PROGRAM
    if _PROGRAM is None:
        _PROGRAM = build_program()
    return _PROGRAM


def _host_inputs(x, cos, sin, Wq, Wk, Wv, Wproj, w_braid):
    cos2 = cos[:, 0, :].astype(np.float32)   # [T, 32]
    sin2 = sin[:, 0, :].astype(np.float32)
    wb = w_braid.astype(np.float32)
    g64 = np.empty((64, T), np.float32)
    g64[:32] = wb[:32, None] * cos2.T - wb[32:, None] * sin2.T
    g64[32:] = wb[32:, None] * cos2.T + wb[:32, None] * sin2.T
    gm = np.concatenate([g64, g64], axis=0)
    mh1 = np.sqrt(cos2.T ** 2 + sin2.T ** 2).astype(np.float32)  # [32, T]
    mh64 = np.concatenate([mh1, mh1], axis=0)
    mh = np.concatenate([mh64, mh64], axis=0)

    sel = np.zeros((128, 3, 6), np.float32)
    sel[0:64, 0, 2] = 1.0
    sel[64:128, 0, 3] = 1.0
    sel[0:64, 1, 4] = 1.0
    sel[64:128, 1, 5] = 1.0
    sel[0:64, 2, 0] = 1.0
    sel[64:128, 2, 1] = 1.0

    tri = (np.arange(128)[None, :] >= np.arange(128)[:, None]).astype(np.float32)
    trz = np.concatenate([np.zeros((128, 384), np.float32), tri], axis=1)
    idn = np.eye(128, dtype=np.float32)
    ones = np.ones((1, T), np.float32)
    pscale = np.float32(1.0 / (T ** 0.5 + 1e-6))

    in_maps = []
    for c in range(NCORES):
        b, g = c // 4, c % 4
        in_maps.append({
            "xT": np.ascontiguousarray(x[b].T),
            "wq": np.ascontiguousarray(Wq[256 * g:256 * (g + 1)].T),
            "wk": np.ascontiguousarray(Wk[128 * g:128 * (g + 1)].T),
            "wv": np.ascontiguousarray(Wv[128 * g:128 * (g + 1)].T),
            "wp": np.ascontiguousarray((Wproj[:, 256 * g:256 * (g + 1)] * pscale).T),
            "gm": gm, "mh": mh, "sel": sel, "trz": trz, "idn": idn, "ones": ones,
            "onz": np.concatenate([np.zeros((1, 128), np.float32),
                                   np.ones((1, 128), np.float32)], axis=0),
        })
    return in_maps


def kernel(x, cos, sin, Wq, Wk, Wv, Wproj, w_braid):
    x = np.asarray(x, np.float32)
    nc = _get_program()
    in_maps = _host_inputs(np.asarray(x, np.float32), np.asarray(cos), np.asarray(sin),
                           np.asarray(Wq, np.float32), np.asarray(Wk, np.float32),
                           np.asarray(Wv, np.float32), np.asarray(Wproj, np.float32),
                           np.asarray(w_braid, np.float32))
    res = run_bass_kernel_spmd(nc, in_maps, list(range(NCORES)))
    out = np.zeros((2, T, C), np.float32)
    for c in range(NCORES):
        out[c // 4] += res.results[c]["outp"]
        out[c // 4] += res.results[c]["outp1"]
    return out



# revision 3
# speedup vs baseline: 1.1682x; 1.1682x over previous
"""Braid causal self-attention Trainium2 kernel (8-core SPMD).

Sharding: data-parallel over batch (2) x tensor-parallel over head groups (4).
Core c handles batch b=c//4, q-heads [4g:4g+4], kv-heads [2g:2g+2], g=c%4.
Each core computes a partial projection output (Wproj input-dim shard);
partials are summed on the host (bf16 partials, fp32 host sum).

Key algebraic restructurings (validated vs reference in fp64):
  - q/k are only needed through the braid scores s_q/s_k. With
    g[d,t] = braid/rotary-folded weights and msq[d,t] = cos^2+sin^2,
    s = (sum_d q[d,t]*g[d,t]) * rsqrt(mean_d q[d,t]^2*msq[d,t] + eps),
    so rotary+rmsnorm are never materialized.
  - attn = sigmoid(s_k[j] + s_q[i]): s_q is broadcast across partitions
    per 1024-query window with a K=2 matmul, the per-key-block s_k column
    rides as the ACT bias, and the two q-heads sharing a kv head are
    stacked side by side so one sigmoid call covers both (half the ACT
    call overhead). Causal masking is a triangular multiply on the
    diagonal strip only.
  - All large matmuls (projections, attn@v, output projection) run in
    bf16 (2x PE rate); the braid score path stays fp32/f32r.
"""
import numpy as np
from contextlib import ExitStack

import ml_dtypes

import concourse.bass as bass
import concourse.mybir as mybir
import concourse.tile as tile
from concourse import bacc
from concourse.bass_utils import run_bass_kernel_spmd

F32 = mybir.dt.float32
F32R = mybir.dt.float32r
BF16 = mybir.dt.bfloat16
AF = mybir.ActivationFunctionType

T = 2048
C = 1024
D = 64
EPS = 1e-6
NCORES = 8


def build_program():
    nc = bacc.Bacc()
    dp = nc.declare_dram_parameter
    xT_d = dp("xT", [C, T], BF16, isOutput=False)         # x[b].T
    wq_d = dp("wq", [C, 256], BF16, isOutput=False)       # Wq[group].T
    wk_d = dp("wk", [C, 128], BF16, isOutput=False)
    wv_d = dp("wv", [C, 128], BF16, isOutput=False)
    wp_d = dp("wp", [256, C], BF16, isOutput=False)       # Wproj[:, group].T (prescaled)
    gm_d = dp("gm", [128, T], F32, isOutput=False)        # braid g (2-head dup)
    msq_d = dp("msq", [128, T], F32, isOutput=False)      # cos^2+sin^2 (2-head dup)
    sel_d = dp("sel", [128, 3, 6], F32, isOutput=False)   # head selector masks
    trz_d = dp("trz", [128, 512], BF16, isOutput=False)   # [zeros(384) | tri(i>=j)]
    idn_d = dp("idn", [128, 128], BF16, isOutput=False)
    ones_d = dp("ones", [1, T], F32, isOutput=False)
    onz_d = dp("onz", [2, 128], F32, isOutput=False)      # [zeros; ones] K=2 bcast pair
    out_d = dp("outp", [T, C], BF16, isOutput=True)
    out1_d = dp("outp1", [T, C], BF16, isOutput=True)

    with tile.TileContext(nc) as tc, \
         nc.allow_low_precision("bf16 matmuls fit the 2e-2 tolerance; score path stays fp32"), \
         ExitStack() as ctx:
        cons = ctx.enter_context(tc.tile_pool(name="cons", bufs=1))
        work = ctx.enter_context(tc.tile_pool(name="work", bufs=1))

        # ---- constants / weights in SBUF ----
        wq_s = cons.tile([128, 8, 256], BF16)
        wk_s = cons.tile([128, 8, 128], BF16)
        wv_s = cons.tile([128, 8, 128], BF16)
        wp_s = cons.tile([128, 2, C], BF16)
        sel_s = cons.tile([128, 3, 6], F32R)
        trz_s = cons.tile([128, 512], BF16)
        idn_s = cons.tile([128, 128], BF16)
        eps_t = cons.tile([128, 1], F32)
        nc.sync.dma_start(out=wq_s[:], in_=wq_d.ap().rearrange("(kt p) m -> p kt m", p=128))
        nc.sync.dma_start(out=wk_s[:], in_=wk_d.ap().rearrange("(kt p) m -> p kt m", p=128))
        nc.sync.dma_start(out=wv_s[:], in_=wv_d.ap().rearrange("(kt p) m -> p kt m", p=128))
        nc.sync.dma_start(out=wp_s[:], in_=wp_d.ap().rearrange("(kt p) m -> p kt m", p=128))
        nc.sync.dma_start(out=sel_s[:], in_=sel_d.ap().bitcast(F32R))
        nc.sync.dma_start(out=trz_s[:], in_=trz_d.ap())
        nc.sync.dma_start(out=idn_s[:], in_=idn_d.ap())
        nc.vector.memset(eps_t[:], EPS)

        # long-lived work tiles
        vT = work.tile([128, T], BF16)
        v_td = work.tile([128, T], BF16)  # 16 blocks of [t128, oc128]
        scomp = work.tile([6, T], F32)
        stil = work.tile([6, T], F32)     # s-tilde accumulator (pre-rsqrt)
        kcolT = work.tile([128, 2, 16], F32)   # s_k columns: [j, kh, jb]
        r1 = work.tile([6, T], F32)
        rq = work.tile([6, T], F32)
        yt0 = work.tile([128, T], BF16)  # heads 0,1 output (d-major)
        yt1 = work.tile([128, T], BF16)

        # ==== phase 1: projections with fused braid reductions ====
        # cn-major: for each 512-column chunk, project all four row-tiles
        # (q0, q1, k, v), then immediately compute the braid products from
        # PSUM and fold them into the per-chunk selector matmuls.
        with tc.tile_pool(name="bpool", bufs=2) as bp, \
             tc.tile_pool(name="xpool", bufs=1) as xp, \
             tc.tile_pool(name="pp1", bufs=2, space="PSUM") as pp1, \
             tc.tile_pool(name="pp2", bufs=2, space="PSUM") as pp2:
            gm_s = bp.tile([128, T], F32, tag="gm")
            msq_s = bp.tile([128, T], F32, tag="msq")
            nc.sync.dma_start(out=gm_s[:], in_=gm_d.ap())
            nc.sync.dma_start(out=msq_s[:], in_=msq_d.ap())
            xT_s = xp.tile([128, 8, T], BF16)
            xr = xT_d.ap().rearrange("(kt p) t -> p kt t", p=128)
            for cn in range(4):
                sl = slice(512 * cn, 512 * cn + 512)
                nc.sync.dma_start(out=xT_s[:, :, sl], in_=xr[:, :, sl])

            tiles = [(wq_s, 0, 0), (wq_s, 128, 1), (wk_s, 0, 2), (wv_s, 0, 3)]
            for cn in range(4):
                sl = slice(512 * cn, 512 * cn + 512)
                pss_t = pp2.tile([6, 512], F32, tag="pss")
                psq_t = pp2.tile([6, 512], F32, tag="psq")
                for w_s, oc0, t_i in tiles:
                    ps = pp1.tile([128, 512], F32, tag="pj")
                    for kt in range(8):
                        nc.tensor.matmul(
                            ps[:], w_s[:, kt, oc0:oc0 + 128],
                            xT_s[:, kt, sl],
                            start=(kt == 0), stop=(kt == 7))
                    if t_i == 3:
                        nc.vector.tensor_copy(vT[:, sl], ps[:])
                    else:
                        a_t = bp.tile([128, 512], F32R, tag="a")
                        q2_t = bp.tile([128, 512], F32, tag="q2")
                        b2_t = bp.tile([128, 512], F32R, tag="b2")
                        nc.vector.tensor_mul(a_t[:], ps[:], gm_s[:, sl])
                        nc.scalar.activation(q2_t[:], ps[:], AF.Square)
                        nc.vector.tensor_mul(b2_t[:], q2_t[:], msq_s[:, sl])
                        nc.tensor.matmul(pss_t[:], sel_s[:, t_i, :], a_t[:],
                                         start=(t_i == 0), stop=(t_i == 2))
                        nc.tensor.matmul(psq_t[:], sel_s[:, t_i, :], b2_t[:],
                                         start=(t_i == 0), stop=(t_i == 2))
                # stash s-tilde and ln(ss/64 + eps) for this chunk
                nc.vector.tensor_copy(stil[:, sl], pss_t[:])
                nc.scalar.activation(r1[:, sl], psq_t[:], AF.Ln,
                                     bias=eps_t[0:6], scale=1.0 / 64.0)

            # v transposes: 16 x [128,128] -> v_td blocks
            for grp in range(4):
                ps_t = pp1.tile([128, 512], BF16, tag="vtp")
                for k in range(4):
                    jb = 4 * grp + k
                    nc.tensor.transpose(
                        ps_t[:, 128 * k:128 * k + 128],
                        vT[:, 128 * jb:128 * jb + 128], idn_s[:])
                nc.vector.tensor_copy(v_td[:, 512 * grp:512 * grp + 512], ps_t[:])

        # rsqrt via exp(-0.5*ln(.)) and final braid scores
        nc.scalar.activation(rq[:], r1[:], AF.Exp, scale=-0.5)
        nc.vector.tensor_mul(scomp[:], stil[:], rq[:])

        # kcolT: s_k columns via a DRAM bounce (free transpose in the APs)
        ksc_d = nc.dram_tensor("kscratch", [2, T], F32)
        nc.sync.dma_start(out=ksc_d.ap(), in_=scomp[0:2, :])
        nc.sync.dma_start(
            out=kcolT[:],
            in_=ksc_d.ap().rearrange("r (b j) -> j r b", j=128))

        # ================= phase 3: attention =================
        with tc.tile_pool(name="svpool", bufs=1) as svp, \
             tc.tile_pool(name="atpool", bufs=4) as atp, \
             tc.tile_pool(name="pp3s", bufs=1, space="PSUM") as pp3s, \
             tc.tile_pool(name="pp3y", bufs=2, space="PSUM") as pp3y:
            # sv: blocks 0-3 = {ones; s_q[h]}; zero/one pair at [4T, 4T+128)
            # for the s_q broadcast matmul
            sv = svp.tile([2, 4 * T + 128], F32R)
            nc.sync.dma_start(out=sv[0:1, 0:4 * T],
                              in_=ones_d.ap().to_broadcast((4, T)).bitcast(F32R))
            nc.sync.dma_start(out=sv[1:2, 0:4 * T], in_=scomp[2:6, :].bitcast(F32R))
            nc.sync.dma_start(out=sv[0:2, 4 * T:4 * T + 128], in_=onz_d.ap().bitcast(F32R))

            # at-pool buffers hold garbage on first use; a stale NaN would
            # survive the 0-multiply mask, so clear them once.
            for _ in range(4):
                at_z = atp.tile([128, 2, 1024], BF16, tag="att", name="at_z")
                nc.vector.memset(at_z[:], 0.0)

            for kh in range(2):
                h0 = 2 * kh
                for hs in (0, 1024):
                    # s_q broadcast for both heads of the pair:
                    # sqb[:, hh, i] = s_q[head h0+hh][hs+i]
                    sqb = pp3s.tile([128, 2, 1024], F32, tag="sqb")
                    for hh in range(2):
                        qbase = T * (h0 + hh)
                        for off in (0, 512):
                            nc.tensor.matmul(
                                sqb[:, hh, off:off + 512],
                                sv[0:2, 4 * T:4 * T + 128],
                                sv[0:2, qbase + hs + off:qbase + hs + off + 512],
                                start=True, stop=True)
                    jmax = (hs + 1024) // 128
                    # 512-aligned windows (matmuls write full 512-wide psum
                    # bank windows; accumulation requires alignment)
                    first_w = [None] * 2
                    last_w = [None] * 2
                    spans = {}
                    for jb in range(jmax):
                        ws = (max(hs, 128 * jb) // 512) * 512
                        spans[jb] = ws
                        for ck in range((ws - hs) // 512, 2):
                            if first_w[ck] is None:
                                first_w[ck] = jb
                            last_w[ck] = jb
                    y_ps = pp3y.tile([128, 1024], F32, tag="yps")
                    for jb in range(jmax):
                        ws = spans[jb]
                        W = hs + 1024 - ws
                        vstart = max(hs, 128 * jb)
                        at_t = atp.tile([128, 2, 1024], BF16, tag="att")
                        # attn = sigmoid(s_q[i] + s_k[j]) for both heads in
                        # one call; s_k column as ACT bias. Only the causal
                        # width is computed; the [ws, vstart) strip holds
                        # stale-but-finite data that the mask zeroes.
                        nc.scalar.activation(
                            at_t[:, :, vstart - ws:W],
                            sqb[:, :, vstart - hs:1024],
                            AF.Sigmoid, bias=kcolT[:, kh, jb:jb + 1])
                        # causal mask: zero the sub-diagonal strip and apply
                        # the triangular mask on the diagonal block in one
                        # multiply against [zeros(384) | tri]
                        strip = 128 * jb - ws
                        mw = strip + (128 if 128 * jb >= hs else 0)
                        if mw > 0:
                            for hh in range(2):
                                nc.vector.tensor_mul(
                                    at_t[:, hh, 0:mw], at_t[:, hh, 0:mw],
                                    trz_s[:, 384 - strip:384 - strip + mw])
                        # attn @ v accumulation (512-wide, bank-aligned);
                        # head hh lands on psum partitions [64*hh, 64*hh+64)
                        for off in range(0, W, 512):
                            ck = (ws - hs + off) // 512
                            for hh in range(2):
                                nc.tensor.matmul(
                                    y_ps[64 * hh:64 * hh + 64,
                                         ws - hs + off:ws - hs + off + 512],
                                    v_td[:, 128 * jb + 64 * kh:128 * jb + 64 * kh + 64],
                                    at_t[:, hh, off:off + 512],
                                    start=(first_w[ck] == jb),
                                    stop=(last_w[ck] == jb))
                    yt_dst = yt0 if kh == 0 else yt1
                    nc.vector.tensor_copy(yt_dst[:, hs:hs + 1024], y_ps[:])

        # ================= phase 4: output projection =================
        # split by K-half: the yt0 half is emitted right after heads 0-1
        # finish, overlapping with heads 2-3 attention; halves summed on host
        with tc.tile_pool(name="ostage", bufs=4) as osp, \
             tc.tile_pool(name="pp4", bufs=4, space="PSUM") as pp4:
            for k2, (yt_src, od) in enumerate([(yt0, out_d), (yt1, out1_d)]):
                for tt in range(16):
                    for cn in range(2):
                        ps_o = pp4.tile([128, 512], F32, tag="opj")
                        nc.tensor.matmul(ps_o[:],
                                         yt_src[:, 128 * tt:128 * tt + 128],
                                         wp_s[:, k2, 512 * cn:512 * cn + 512],
                                         start=True, stop=True)
                        o_t = osp.tile([128, 512], BF16, tag="ost")
                        nc.vector.tensor_copy(o_t[:], ps_o[:])
                        nc.sync.dma_start(
                            out=od.ap()[128 * tt:128 * tt + 128, 512 * cn:512 * cn + 512],
                            in_=o_t[:])

    nc.compile()
    return nc


_PROGRAM = None


def _get_program():
    global _PROGRAM
    if _PROGRAM is None:
        _PROGRAM = build_program()
    return _PROGRAM


def _host_inputs(x, cos, sin, Wq, Wk, Wv, Wproj, w_braid):
    bf = ml_dtypes.bfloat16
    cos2 = cos[:, 0, :].astype(np.float32)   # [T, 32]
    sin2 = sin[:, 0, :].astype(np.float32)
    wb = w_braid.astype(np.float32)
    g64 = np.empty((64, T), np.float32)
    g64[:32] = wb[:32, None] * cos2.T - wb[32:, None] * sin2.T
    g64[32:] = wb[32:, None] * cos2.T + wb[:32, None] * sin2.T
    gm = np.concatenate([g64, g64], axis=0)
    msq1 = (cos2.T ** 2 + sin2.T ** 2).astype(np.float32)  # [32, T]
    msq64 = np.concatenate([msq1, msq1], axis=0)
    msq = np.concatenate([msq64, msq64], axis=0)

    sel = np.zeros((128, 3, 6), np.float32)
    sel[0:64, 0, 2] = 1.0
    sel[64:128, 0, 3] = 1.0
    sel[0:64, 1, 4] = 1.0
    sel[64:128, 1, 5] = 1.0
    sel[0:64, 2, 0] = 1.0
    sel[64:128, 2, 1] = 1.0

    tri = (np.arange(128)[None, :] >= np.arange(128)[:, None]).astype(bf)
    trz = np.concatenate([np.zeros((128, 384), bf), tri], axis=1)
    idn = np.eye(128, dtype=bf)
    ones = np.ones((1, T), np.float32)
    pscale = np.float32(1.0 / (T ** 0.5 + 1e-6))

    in_maps = []
    for c in range(NCORES):
        b, g = c // 4, c % 4
        in_maps.append({
            "xT": np.ascontiguousarray(x[b].T).astype(bf),
            "wq": np.ascontiguousarray(Wq[256 * g:256 * (g + 1)].T).astype(bf),
            "wk": np.ascontiguousarray(Wk[128 * g:128 * (g + 1)].T).astype(bf),
            "wv": np.ascontiguousarray(Wv[128 * g:128 * (g + 1)].T).astype(bf),
            "wp": np.ascontiguousarray((Wproj[:, 256 * g:256 * (g + 1)] * pscale).T).astype(bf),
            "gm": gm, "msq": msq, "sel": sel, "trz": trz, "idn": idn, "ones": ones,
            "onz": np.concatenate([np.zeros((1, 128), np.float32),
                                   np.ones((1, 128), np.float32)], axis=0),
        })
    return in_maps


def kernel(x, cos, sin, Wq, Wk, Wv, Wproj, w_braid):
    x = np.asarray(x, np.float32)
    nc = _get_program()
    in_maps = _host_inputs(np.asarray(x, np.float32), np.asarray(cos), np.asarray(sin),
                           np.asarray(Wq, np.float32), np.asarray(Wk, np.float32),
                           np.asarray(Wv, np.float32), np.asarray(Wproj, np.float32),
                           np.asarray(w_braid, np.float32))
    res = run_bass_kernel_spmd(nc, in_maps, list(range(NCORES)))
    out = np.zeros((2, T, C), np.float32)
    for c in range(NCORES):
        out[c // 4] += res.results[c]["outp"].astype(np.float32)
        out[c // 4] += res.results[c]["outp1"].astype(np.float32)
    return out


# revision 10
# speedup vs baseline: 1.4172x; 1.2131x over previous
"""Braid causal self-attention Trainium2 kernel (8-core SPMD).

Sharding: data-parallel over batch (2) x tensor-parallel over head groups (4).
Core c handles batch b=c//4, q-heads [4g:4g+4], kv-heads [2g:2g+2], g=c%4.
Each core computes a partial projection output (Wproj input-dim shard);
partials are summed on the host (bf16 partials, fp32 host sum).

Key structure (v3):
  - q/k are only needed through the braid scores s_q/s_k: with
    g[d,t] = braid/rotary-folded weights and mh[d,t] = sqrt(cos^2+sin^2),
    s = (sum_d q*g) * rsqrt(mean_d (q*mh)^2 + eps); rotary+rmsnorm are
    never materialized. The per-chunk score tail (Ln/Exp rsqrt, DRAM
    bounce) is pipelined with the projections.
  - attn = sigmoid(s_q[i] + s_k[j]): s_q rows are partition-broadcast by
    DMA from a DRAM bounce (no matmul, no psum), the per-key-block s_k
    column rides as the ACT bias, and the two q-heads sharing a kv head
    are stacked so one sigmoid call covers both. Causal masking is a
    128-wide triangular multiply on diagonal blocks only; attn@v uses
    partial-width matmuls so sub-diagonal strips are never touched.
  - All large matmuls (projections, attn@v, output projection) run in
    bf16; the braid score path stays fp32/f32r. v is transposed with the
    DMA transpose XBAR. Output projection is emitted in quarters as yt
    halves complete, overlapping the attention phase.
"""
import numpy as np
from contextlib import ExitStack

import ml_dtypes

import concourse.bass as bass
import concourse.mybir as mybir
import concourse.tile as tile
from concourse import bacc
from concourse.bass_utils import run_bass_kernel_spmd

F32 = mybir.dt.float32
F32R = mybir.dt.float32r
BF16 = mybir.dt.bfloat16
AF = mybir.ActivationFunctionType

T = 2048
C = 1024
D = 64
EPS = 1e-6
NCORES = 8


def build_program():
    nc = bacc.Bacc()
    dp = nc.declare_dram_parameter
    xT_d = dp("xT", [C, T], BF16, isOutput=False)         # x[b].T
    wq_d = dp("wq", [C, 256], BF16, isOutput=False)       # Wq[group].T
    wk_d = dp("wk", [C, 128], BF16, isOutput=False)
    wv_d = dp("wv", [C, 128], BF16, isOutput=False)
    wp_d = dp("wp", [256, C], BF16, isOutput=False)       # Wproj[:, group].T (prescaled)
    gm_d = dp("gm", [128, T], F32, isOutput=False)        # braid g (2-head dup)
    mh_d = dp("mh", [128, T], F32, isOutput=False)        # sqrt(cos^2+sin^2) (2-head dup)
    sel_d = dp("sel", [128, 3, 6], F32, isOutput=False)   # head selector masks
    tri_d = dp("tri", [128, 128], BF16, isOutput=False)   # tri(i>=j)
    out_d = dp("outp", [T, C], BF16, isOutput=True)
    out1_d = dp("outp1", [T, C], BF16, isOutput=True)

    with tile.TileContext(nc) as tc, \
         nc.allow_low_precision("bf16 matmuls fit the 2e-2 tolerance; score path stays fp32"), \
         ExitStack() as ctx:
        cons = ctx.enter_context(tc.tile_pool(name="cons", bufs=1))
        work = ctx.enter_context(tc.tile_pool(name="work", bufs=1))

        # ---- constants / weights in SBUF (DMAs spread across queues) ----
        wq_s = cons.tile([128, 8, 256], BF16)
        wk_s = cons.tile([128, 8, 128], BF16)
        wv_s = cons.tile([128, 8, 128], BF16)
        wp_s = cons.tile([128, 2, C], BF16)
        sel_s = cons.tile([128, 3, 6], F32R)
        tri_s = cons.tile([128, 128], BF16)
        eps_t = cons.tile([128, 1], F32)
        gm_s = cons.tile([128, T], F32)
        mh_s = cons.tile([128, T], F32)
        xT_s = cons.tile([128, 8, T], BF16)
        xr = xT_d.ap().rearrange("(kt p) t -> p kt t", p=128)
        nc.sync.dma_start(out=xT_s[:, :, 0:512], in_=xr[:, :, 0:512])
        nc.scalar.dma_start(out=wq_s[:], in_=wq_d.ap().rearrange("(kt p) m -> p kt m", p=128))
        nc.scalar.dma_start(out=wk_s[:], in_=wk_d.ap().rearrange("(kt p) m -> p kt m", p=128))
        nc.scalar.dma_start(out=wv_s[:], in_=wv_d.ap().rearrange("(kt p) m -> p kt m", p=128))
        nc.gpsimd.dma_start(out=xT_s[:, :, 512:1024], in_=xr[:, :, 512:1024])
        nc.scalar.dma_start(out=gm_s[:], in_=gm_d.ap())
        nc.gpsimd.dma_start(out=mh_s[:], in_=mh_d.ap())
        nc.sync.dma_start(out=xT_s[:, :, 1024:1536], in_=xr[:, :, 1024:1536])
        nc.gpsimd.dma_start(out=wp_s[:], in_=wp_d.ap().rearrange("(kt p) m -> p kt m", p=128))
        nc.scalar.dma_start(out=sel_s[:], in_=sel_d.ap().bitcast(F32R))
        nc.scalar.dma_start(out=tri_s[:], in_=tri_d.ap())
        nc.gpsimd.dma_start(out=xT_s[:, :, 1536:2048], in_=xr[:, :, 1536:2048])
        nc.vector.memset(eps_t[:], EPS)

        # long-lived work tiles
        vT = work.tile([128, T], BF16)
        v_td = work.tile([128, T], BF16)  # 16 blocks of [t128, oc128]
        scomp = work.tile([6, T], F32)
        stil = work.tile([6, T], F32)     # s-tilde accumulator (pre-rsqrt)
        kcolA = work.tile([128, 2, 8], F32)   # s_k columns: [j, kh, jb] jb 0-7
        kcolB = work.tile([128, 2, 8], F32)   # s_k columns jb 8-15
        r1 = work.tile([6, T], F32)
        rq = work.tile([6, T], F32)
        yt0 = work.tile([128, T], BF16)  # heads 0,1 output (d-major)
        yt1 = work.tile([128, T], BF16)

        ksc0_d = nc.dram_tensor("kscratch0", [2, 1024], F32)
        ksc1_d = nc.dram_tensor("kscratch1", [2, 1024], F32)
        sq_d = nc.dram_tensor("sqscratch", [4, T], F32)

        # ==== phase 1: projections with fused braid reductions ====
        # cn-major: for each 512-column chunk, project all four row-tiles
        # (q0, q1, k, v), compute the braid products from PSUM, fold them
        # into the per-chunk selector matmuls, and finish the chunk's
        # scores (rsqrt + DRAM bounce) so attention can start early.
        with tc.tile_pool(name="bpool", bufs=2) as bp, \
             tc.tile_pool(name="pp1", bufs=2, space="PSUM") as pp1, \
             tc.tile_pool(name="pp2", bufs=2, space="PSUM") as pp2:
            tiles = [(wq_s, 0, 0), (wq_s, 128, 1), (wk_s, 0, 2), (wv_s, 0, 3)]
            for cn in range(4):
                sl = slice(512 * cn, 512 * cn + 512)
                pss_t = pp2.tile([6, 512], F32, tag="pss")
                psq_t = pp2.tile([6, 512], F32, tag="psq")
                for w_s, oc0, t_i in tiles:
                    ps = pp1.tile([128, 512], F32, tag="pj")
                    for kt in range(8):
                        nc.tensor.matmul(
                            ps[:], w_s[:, kt, oc0:oc0 + 128],
                            xT_s[:, kt, sl],
                            start=(kt == 0), stop=(kt == 7))
                    if t_i == 3:
                        nc.vector.tensor_copy(vT[:, sl], ps[:])
                    else:
                        a_t = bp.tile([128, 512], F32R, tag="a")
                        b_t = bp.tile([128, 512], F32, tag="b")
                        b2_t = bp.tile([128, 512], F32R, tag="b2")
                        nc.vector.tensor_mul(a_t[:], ps[:], gm_s[:, sl])
                        nc.vector.tensor_mul(b_t[:], ps[:], mh_s[:, sl])
                        nc.gpsimd.tensor_mul(b2_t[:], b_t[:], b_t[:])
                        nc.tensor.matmul(pss_t[:], sel_s[:, t_i, :], a_t[:],
                                         start=(t_i == 0), stop=(t_i == 2))
                        nc.tensor.matmul(psq_t[:], sel_s[:, t_i, :], b2_t[:],
                                         start=(t_i == 0), stop=(t_i == 2))
                # per-chunk score tail: s = stil * exp(-0.5*ln(ssq/64+eps))
                nc.vector.tensor_copy(stil[:, sl], pss_t[:])
                nc.scalar.activation(r1[:, sl], psq_t[:], AF.Ln,
                                     bias=eps_t[0:6], scale=1.0 / 64.0)
                nc.scalar.activation(rq[:, sl], r1[:, sl], AF.Exp, scale=-0.5)
                nc.vector.tensor_mul(scomp[:, sl], stil[:, sl], rq[:, sl])
                kd = ksc0_d if cn < 2 else ksc1_d
                nc.sync.dma_start(out=kd.ap()[:, 512 * (cn % 2):512 * (cn % 2) + 512],
                                  in_=scomp[0:2, sl])
                nc.sync.dma_start(out=sq_d.ap()[:, sl], in_=scomp[2:6, sl])
                if cn in (1, 3):
                    kt_dst = kcolA if cn == 1 else kcolB
                    kt_src = ksc0_d if cn == 1 else ksc1_d
                    nc.gpsimd.dma_start(
                        out=kt_dst[:],
                        in_=kt_src.ap().rearrange("r (b j) -> j r b", j=128))
                # v transpose for this chunk via the DMA transpose XBAR
                for k in range(4):
                    jb = 4 * cn + k
                    qd = [nc.sync, nc.scalar][k % 2]
                    qd.dma_start(out=v_td[:, 128 * jb:128 * jb + 128],
                                 in_=vT[:, 128 * jb:128 * jb + 128],
                                 transpose=True)

        # ================= phase 3: attention + streamed projection ======
        with tc.tile_pool(name="sqpool", bufs=3) as sqp, \
             tc.tile_pool(name="atpool", bufs=8) as atp, \
             tc.tile_pool(name="ostage", bufs=4) as osp, \
             tc.tile_pool(name="pp3y", bufs=3, space="PSUM") as pp3y, \
             tc.tile_pool(name="pp4", bufs=2, space="PSUM") as pp4:
            for hs in (0, 1024):
                for kh in range(2):
                    h0 = 2 * kh
                    # s_q rows for both heads, partition-broadcast by DMA
                    sqb = sqp.tile([128, 2, 1024], F32, tag="sqb")
                    for hh in range(2):
                        qd = [nc.sync, nc.gpsimd][hh]
                        qd.dma_start(
                            out=sqb[:, hh, :],
                            in_=sq_d.ap()[h0 + hh:h0 + hh + 1, hs:hs + 1024]
                                .to_broadcast((128, 1024)))
                    jmax = (hs + 1024) // 128
                    y_ps = pp3y.tile([128, 1024], F32, tag="yps")
                    # last jb touching each 512-wide psum window
                    last_w = [min((hs + 512 * ck + 512) // 128, jmax) - 1
                              for ck in range(2)]
                    for jb in range(jmax):
                        vstart = max(hs, 128 * jb)
                        voff = vstart - hs   # first live col within the window
                        at_t = atp.tile([128, 2, 1024], BF16, tag="att")
                        # attn = sigmoid(s_q[i] + s_k[j]) for both heads in
                        # one call; s_k column as ACT bias.
                        nc.scalar.activation(
                            at_t[:, :, voff:1024],
                            sqb[:, :, voff:1024],
                            AF.Sigmoid,
                            bias=(kcolA if jb < 8 else kcolB)[:, kh, jb % 8:jb % 8 + 1])
                        # causal tri mask on the diagonal block only
                        if 128 * jb >= hs:
                            for hh in range(2):
                                nc.vector.tensor_mul(
                                    at_t[:, hh, voff:voff + 128],
                                    at_t[:, hh, voff:voff + 128],
                                    tri_s[:])
                        # attn @ v accumulation: partial-width matmuls start
                        # at the causal boundary; head hh lands on psum
                        # partitions [64*hh, 64*hh+64)
                        for ck in range(voff // 512, 2):
                            lo = max(voff, 512 * ck)
                            for hh in range(2):
                                nc.tensor.matmul(
                                    y_ps[64 * hh:64 * hh + 64, lo:512 * ck + 512],
                                    v_td[:, 128 * jb + 64 * kh:128 * jb + 64 * kh + 64],
                                    at_t[:, hh, lo:512 * ck + 512],
                                    start=(jb == 0),
                                    stop=(last_w[ck] == jb))
                    yt_dst = yt0 if kh == 0 else yt1
                    nc.vector.tensor_copy(yt_dst[:, hs:hs + 1024], y_ps[:])

                    # stream out the finished quarter of the output
                    # projection (yt[kh][:, hs:hs+1024] is now complete)
                    od = out_d if kh == 0 else out1_d
                    yt_src = yt_dst
                    tail = hs == 1024 and kh == 1
                    for ti in range(8):
                        tt = hs // 128 + ti
                        o_t = osp.tile([128, C], BF16, tag="ost")
                        for cn in range(2):
                            ps_o = pp4.tile([128, 512], F32, tag="opj")
                            nc.tensor.matmul(ps_o[:],
                                             yt_src[:, 128 * tt:128 * tt + 128],
                                             wp_s[:, kh, 512 * cn:512 * cn + 512],
                                             start=True, stop=True)
                            if tail and cn == 1:
                                nc.scalar.copy(o_t[:, 512 * cn:512 * cn + 512], ps_o[:])
                            else:
                                nc.vector.tensor_copy(o_t[:, 512 * cn:512 * cn + 512], ps_o[:])
                        qd = [nc.sync, nc.gpsimd][ti % 2]
                        qd.dma_start(
                            out=od.ap()[128 * tt:128 * tt + 128, :],
                            in_=o_t[:])

    nc.compile()
    return nc


_PROGRAM = None


def _get_program():
    global _PROGRAM
    if _PROGRAM is None:
        _PROGRAM = build_program()
    return _PROGRAM


def _host_inputs(x, cos, sin, Wq, Wk, Wv, Wproj, w_braid):
    bf = ml_dtypes.bfloat16
    cos2 = cos[:, 0, :].astype(np.float32)   # [T, 32]
    sin2 = sin[:, 0, :].astype(np.float32)
    wb = w_braid.astype(np.float32)
    g64 = np.empty((64, T), np.float32)
    g64[:32] = wb[:32, None] * cos2.T - wb[32:, None] * sin2.T
    g64[32:] = wb[32:, None] * cos2.T + wb[:32, None] * sin2.T
    gm = np.concatenate([g64, g64], axis=0)
    mh1 = np.sqrt(cos2.T ** 2 + sin2.T ** 2).astype(np.float32)  # [32, T]
    mh64 = np.concatenate([mh1, mh1], axis=0)
    mh = np.concatenate([mh64, mh64], axis=0)

    sel = np.zeros((128, 3, 6), np.float32)
    sel[0:64, 0, 2] = 1.0
    sel[64:128, 0, 3] = 1.0
    sel[0:64, 1, 4] = 1.0
    sel[64:128, 1, 5] = 1.0
    sel[0:64, 2, 0] = 1.0
    sel[64:128, 2, 1] = 1.0

    tri = (np.arange(128)[None, :] >= np.arange(128)[:, None]).astype(bf)
    pscale = np.float32(1.0 / (T ** 0.5 + 1e-6))

    in_maps = []
    for c in range(NCORES):
        b, g = c // 4, c % 4
        in_maps.append({
            "xT": np.ascontiguousarray(x[b].T).astype(bf),
            "wq": np.ascontiguousarray(Wq[256 * g:256 * (g + 1)].T).astype(bf),
            "wk": np.ascontiguousarray(Wk[128 * g:128 * (g + 1)].T).astype(bf),
            "wv": np.ascontiguousarray(Wv[128 * g:128 * (g + 1)].T).astype(bf),
            "wp": np.ascontiguousarray((Wproj[:, 256 * g:256 * (g + 1)] * pscale).T).astype(bf),
            "gm": gm, "mh": mh, "sel": sel, "tri": tri,
        })
    return in_maps


def kernel(x, cos, sin, Wq, Wk, Wv, Wproj, w_braid):
    x = np.asarray(x, np.float32)
    nc = _get_program()
    in_maps = _host_inputs(np.asarray(x, np.float32), np.asarray(cos), np.asarray(sin),
                           np.asarray(Wq, np.float32), np.asarray(Wk, np.float32),
                           np.asarray(Wv, np.float32), np.asarray(Wproj, np.float32),
                           np.asarray(w_braid, np.float32))
    res = run_bass_kernel_spmd(nc, in_maps, list(range(NCORES)))
    out = np.zeros((2, T, C), np.float32)
    for c in range(NCORES):
        out[c // 4] += res.results[c]["outp"].astype(np.float32)
        out[c // 4] += res.results[c]["outp1"].astype(np.float32)
    return out


# revision 11
# speedup vs baseline: 1.4591x; 1.0296x over previous
"""Braid causal self-attention Trainium2 kernel (8-core SPMD).

Sharding: data-parallel over batch (2) x tensor-parallel over head groups (4).
Core c handles batch b=c//4, q-heads [4g:4g+4], kv-heads [2g:2g+2], g=c%4.
Each core computes a partial projection output (Wproj input-dim shard);
partials are summed on the host (bf16 partials, fp32 host sum).

Key structure (v3):
  - q/k are only needed through the braid scores s_q/s_k: with
    g[d,t] = braid/rotary-folded weights and mh[d,t] = sqrt(cos^2+sin^2),
    s = (sum_d q*g) * rsqrt(mean_d (q*mh)^2 + eps); rotary+rmsnorm are
    never materialized. The per-chunk score tail (Ln/Exp rsqrt, DRAM
    bounce) is pipelined with the projections.
  - attn = sigmoid(s_q[i] + s_k[j]): s_q rows are partition-broadcast by
    DMA from a DRAM bounce (no matmul, no psum), the per-key-block s_k
    column rides as the ACT bias, and the two q-heads sharing a kv head
    are stacked so one sigmoid call covers both. Causal masking is a
    128-wide triangular multiply on diagonal blocks only; attn@v uses
    partial-width matmuls so sub-diagonal strips are never touched.
  - All large matmuls (projections, attn@v, output projection) run in
    bf16; the braid score path stays fp32/f32r. v is transposed with the
    DMA transpose XBAR. Output projection is emitted in quarters as yt
    halves complete, overlapping the attention phase.
"""
import numpy as np
from contextlib import ExitStack

import ml_dtypes

import concourse.bass as bass
import concourse.mybir as mybir
import concourse.tile as tile
from concourse import bacc
from concourse.bass_utils import run_bass_kernel_spmd

F32 = mybir.dt.float32
F32R = mybir.dt.float32r
BF16 = mybir.dt.bfloat16
AF = mybir.ActivationFunctionType

T = 2048
C = 1024
D = 64
EPS = 1e-6
NCORES = 8


def build_program():
    nc = bacc.Bacc()
    dp = nc.declare_dram_parameter
    xT_d = dp("xT", [C, T], BF16, isOutput=False)         # x[b].T
    wq_d = dp("wq", [C, 256], BF16, isOutput=False)       # Wq[group].T
    wk_d = dp("wk", [C, 128], BF16, isOutput=False)
    wv_d = dp("wv", [C, 128], BF16, isOutput=False)
    wp_d = dp("wp", [256, C], BF16, isOutput=False)       # Wproj[:, group].T (prescaled)
    gm_d = dp("gm", [128, T], F32, isOutput=False)        # braid g (2-head dup)
    mh_d = dp("mh", [128, T], F32, isOutput=False)        # sqrt(cos^2+sin^2) (2-head dup)
    sel_d = dp("sel", [128, 3, 6], F32, isOutput=False)   # head selector masks
    tri_d = dp("tri", [128, 128], BF16, isOutput=False)   # tri(i>=j)
    out_d = dp("outp", [T, C], BF16, isOutput=True)
    out1_d = dp("outp1", [T, C], BF16, isOutput=True)

    with tile.TileContext(nc) as tc, \
         nc.allow_low_precision("bf16 matmuls fit the 2e-2 tolerance; score path stays fp32"), \
         ExitStack() as ctx:
        cons = ctx.enter_context(tc.tile_pool(name="cons", bufs=1))
        work = ctx.enter_context(tc.tile_pool(name="work", bufs=1))

        # ---- constants / weights in SBUF (DMAs spread across queues) ----
        wq_s = cons.tile([128, 8, 256], BF16)
        wk_s = cons.tile([128, 8, 128], BF16)
        wv_s = cons.tile([128, 8, 128], BF16)
        wp_s = cons.tile([128, 2, C], BF16)
        sel_s = cons.tile([128, 3, 6], F32R)
        tri_s = cons.tile([128, 128], BF16)
        eps_t = cons.tile([128, 1], F32)
        gm_s = cons.tile([128, T], F32)
        mh_s = cons.tile([128, T], F32)
        xT_s = cons.tile([128, 8, T], BF16)
        xr = xT_d.ap().rearrange("(kt p) t -> p kt t", p=128)
        nc.sync.dma_start(out=xT_s[:, :, 0:512], in_=xr[:, :, 0:512])
        nc.sync.dma_start(out=wq_s[:], in_=wq_d.ap().rearrange("(kt p) m -> p kt m", p=128))
        nc.gpsimd.dma_start(out=wk_s[:], in_=wk_d.ap().rearrange("(kt p) m -> p kt m", p=128))
        nc.gpsimd.dma_start(out=wv_s[:], in_=wv_d.ap().rearrange("(kt p) m -> p kt m", p=128))
        nc.gpsimd.dma_start(out=xT_s[:, :, 512:1024], in_=xr[:, :, 512:1024])
        nc.sync.dma_start(out=gm_s[:], in_=gm_d.ap())
        nc.gpsimd.dma_start(out=mh_s[:], in_=mh_d.ap())
        nc.sync.dma_start(out=xT_s[:, :, 1024:1536], in_=xr[:, :, 1024:1536])
        nc.gpsimd.dma_start(out=wp_s[:], in_=wp_d.ap().rearrange("(kt p) m -> p kt m", p=128))
        nc.sync.dma_start(out=sel_s[:], in_=sel_d.ap().bitcast(F32R))
        nc.sync.dma_start(out=tri_s[:], in_=tri_d.ap())
        nc.gpsimd.dma_start(out=xT_s[:, :, 1536:2048], in_=xr[:, :, 1536:2048])
        nc.vector.memset(eps_t[:], EPS)

        # long-lived work tiles
        vT = work.tile([128, T], BF16)
        v_td = work.tile([128, T], BF16)  # 16 blocks of [t128, oc128]
        scomp = work.tile([6, T], F32)
        stil = work.tile([6, T], F32)     # s-tilde accumulator (pre-rsqrt)
        kcolA = work.tile([128, 2, 8], F32)   # s_k columns: [j, kh, jb] jb 0-7
        kcolB = work.tile([128, 2, 8], F32)   # s_k columns jb 8-15
        r1 = work.tile([6, T], F32)
        psq_s = work.tile([6, T], F32)
        rq = work.tile([6, T], F32)
        yt0 = work.tile([128, T], BF16)  # heads 0,1 output (d-major)
        yt1 = work.tile([128, T], BF16)

        ksc0_d = nc.dram_tensor("kscratch0", [2, 1024], F32)
        ksc1_d = nc.dram_tensor("kscratch1", [2, 1024], F32)
        sq_d = nc.dram_tensor("sqscratch", [4, T], F32)

        # ==== phase 1: projections with fused braid reductions ====
        # cn-major: for each 512-column chunk, project all four row-tiles
        # (q0, q1, k, v), compute the braid products from PSUM, fold them
        # into the per-chunk selector matmuls, and finish the chunk's
        # scores (rsqrt + DRAM bounce) so attention can start early.
        with tc.tile_pool(name="bpool", bufs=2) as bp, \
             tc.tile_pool(name="pp1", bufs=2, space="PSUM") as pp1, \
             tc.tile_pool(name="pp2", bufs=2, space="PSUM") as pp2:
            tiles = [(wq_s, 0, 0), (wq_s, 128, 1), (wk_s, 0, 2), (wv_s, 0, 3)]
            for cn in range(4):
                sl = slice(512 * cn, 512 * cn + 512)
                pss_t = pp2.tile([6, 512], F32, tag="pss")
                psq_t = pp2.tile([6, 512], F32, tag="psq")
                for w_s, oc0, t_i in tiles:
                    ps = pp1.tile([128, 512], F32, tag="pj")
                    for kt in range(8):
                        nc.tensor.matmul(
                            ps[:], w_s[:, kt, oc0:oc0 + 128],
                            xT_s[:, kt, sl],
                            start=(kt == 0), stop=(kt == 7))
                    if t_i == 3:
                        nc.vector.tensor_copy(vT[:, sl], ps[:])
                    else:
                        a_t = bp.tile([128, 512], F32R, tag="a")
                        b_t = bp.tile([128, 512], F32, tag="b")
                        b2_t = bp.tile([128, 512], F32R, tag="b2")
                        nc.vector.tensor_mul(a_t[:], ps[:], gm_s[:, sl])
                        nc.vector.tensor_mul(b_t[:], ps[:], mh_s[:, sl])
                        nc.gpsimd.tensor_mul(b2_t[:], b_t[:], b_t[:])
                        nc.tensor.matmul(pss_t[:], sel_s[:, t_i, :], a_t[:],
                                         start=(t_i == 0), stop=(t_i == 2))
                        nc.tensor.matmul(psq_t[:], sel_s[:, t_i, :], b2_t[:],
                                         start=(t_i == 0), stop=(t_i == 2))
                # per-chunk score tail: s = stil * exp(-0.5*ln(ssq/64+eps)).
                # cn 0/1 run chunk-wise so attention can start early; cn 2/3
                # are merged into one Ln/Exp pair to bound sigmoid-set
                # table thrashing on the ACT engine.
                nc.vector.tensor_copy(stil[:, sl], pss_t[:])
                nc.vector.tensor_copy(psq_s[:, sl], psq_t[:])
                if cn < 2:
                    lsl = sl
                elif cn == 3:
                    lsl = slice(1024, 2048)
                else:
                    lsl = None
                if lsl is not None:
                    nc.scalar.activation(r1[:, lsl], psq_s[:, lsl], AF.Ln,
                                         bias=eps_t[0:6], scale=1.0 / 64.0)
                    nc.scalar.activation(rq[:, lsl], r1[:, lsl], AF.Exp, scale=-0.5)
                    nc.vector.tensor_mul(scomp[:, lsl], stil[:, lsl], rq[:, lsl])
                if lsl is not None:
                    kd = ksc0_d if cn < 2 else ksc1_d
                    ksl = slice(0, 1024) if cn == 3 else slice(512 * (cn % 2), 512 * (cn % 2) + 512)
                    nc.sync.dma_start(out=kd.ap()[:, ksl], in_=scomp[0:2, lsl])
                    nc.sync.dma_start(out=sq_d.ap()[:, lsl], in_=scomp[2:6, lsl])
                if cn in (1, 3):
                    kt_dst = kcolA if cn == 1 else kcolB
                    kt_src = ksc0_d if cn == 1 else ksc1_d
                    nc.gpsimd.dma_start(
                        out=kt_dst[:],
                        in_=kt_src.ap().rearrange("r (b j) -> j r b", j=128))
                # v transpose for this chunk via the DMA transpose XBAR
                for k in range(4):
                    jb = 4 * cn + k
                    nc.sync.dma_start(out=v_td[:, 128 * jb:128 * jb + 128],
                                 in_=vT[:, 128 * jb:128 * jb + 128],
                                 transpose=True)

        # ================= phase 3: attention + streamed projection ======
        with tc.tile_pool(name="sqpool", bufs=3) as sqp, \
             tc.tile_pool(name="atpool", bufs=8) as atp, \
             tc.tile_pool(name="ostage", bufs=4) as osp, \
             tc.tile_pool(name="pp3y", bufs=3, space="PSUM") as pp3y, \
             tc.tile_pool(name="pp4", bufs=2, space="PSUM") as pp4:
            for hs in (0, 1024):
                for kh in range(2):
                    h0 = 2 * kh
                    # s_q rows for both heads, partition-broadcast by DMA
                    sqb = sqp.tile([128, 2, 1024], F32, tag="sqb")
                    for hh in range(2):
                        qd = [nc.sync, nc.gpsimd][hh]
                        qd.dma_start(
                            out=sqb[:, hh, :],
                            in_=sq_d.ap()[h0 + hh:h0 + hh + 1, hs:hs + 1024]
                                .to_broadcast((128, 1024)))
                    jmax = (hs + 1024) // 128
                    y_ps = pp3y.tile([128, 1024], F32, tag="yps")
                    # last jb touching each 512-wide psum window
                    last_w = [min((hs + 512 * ck + 512) // 128, jmax) - 1
                              for ck in range(2)]
                    for jb in range(jmax):
                        vstart = max(hs, 128 * jb)
                        voff = vstart - hs   # first live col within the window
                        at_t = atp.tile([128, 2, 1024], BF16, tag="att")
                        # attn = sigmoid(s_q[i] + s_k[j]) for both heads in
                        # one call; s_k column as ACT bias.
                        nc.scalar.activation(
                            at_t[:, :, voff:1024],
                            sqb[:, :, voff:1024],
                            AF.Sigmoid,
                            bias=(kcolA if jb < 8 else kcolB)[:, kh, jb % 8:jb % 8 + 1])
                        # causal tri mask on the diagonal block only
                        if 128 * jb >= hs:
                            for hh in range(2):
                                nc.vector.tensor_mul(
                                    at_t[:, hh, voff:voff + 128],
                                    at_t[:, hh, voff:voff + 128],
                                    tri_s[:])
                        # attn @ v accumulation: partial-width matmuls start
                        # at the causal boundary; head hh lands on psum
                        # partitions [64*hh, 64*hh+64)
                        for ck in range(voff // 512, 2):
                            lo = max(voff, 512 * ck)
                            for hh in range(2):
                                nc.tensor.matmul(
                                    y_ps[64 * hh:64 * hh + 64, lo:512 * ck + 512],
                                    v_td[:, 128 * jb + 64 * kh:128 * jb + 64 * kh + 64],
                                    at_t[:, hh, lo:512 * ck + 512],
                                    start=(jb == 0),
                                    stop=(last_w[ck] == jb))
                    yt_dst = yt0 if kh == 0 else yt1
                    nc.vector.tensor_copy(yt_dst[:, hs:hs + 1024], y_ps[:])

                    # stream out the finished quarter of the output
                    # projection (yt[kh][:, hs:hs+1024] is now complete)
                    od = out_d if kh == 0 else out1_d
                    yt_src = yt_dst
                    tail = hs == 1024 and kh == 1
                    for ti in range(8):
                        tt = hs // 128 + ti
                        o_t = osp.tile([128, C], BF16, tag="ost")
                        for cn in range(2):
                            ps_o = pp4.tile([128, 512], F32, tag="opj")
                            nc.tensor.matmul(ps_o[:],
                                             yt_src[:, 128 * tt:128 * tt + 128],
                                             wp_s[:, kh, 512 * cn:512 * cn + 512],
                                             start=True, stop=True)
                            if tail and cn == 1:
                                nc.scalar.copy(o_t[:, 512 * cn:512 * cn + 512], ps_o[:])
                            else:
                                nc.vector.tensor_copy(o_t[:, 512 * cn:512 * cn + 512], ps_o[:])
                        qd = [nc.sync, nc.gpsimd][ti % 2]
                        qd.dma_start(
                            out=od.ap()[128 * tt:128 * tt + 128, :],
                            in_=o_t[:])

    nc.compile()
    return nc


_PROGRAM = None


def _get_program():
    global _PROGRAM
    if _PROGRAM is None:
        _PROGRAM = build_program()
    return _PROGRAM


def _host_inputs(x, cos, sin, Wq, Wk, Wv, Wproj, w_braid):
    bf = ml_dtypes.bfloat16
    cos2 = cos[:, 0, :].astype(np.float32)   # [T, 32]
    sin2 = sin[:, 0, :].astype(np.float32)
    wb = w_braid.astype(np.float32)
    g64 = np.empty((64, T), np.float32)
    g64[:32] = wb[:32, None] * cos2.T - wb[32:, None] * sin2.T
    g64[32:] = wb[32:, None] * cos2.T + wb[:32, None] * sin2.T
    gm = np.concatenate([g64, g64], axis=0)
    mh1 = np.sqrt(cos2.T ** 2 + sin2.T ** 2).astype(np.float32)  # [32, T]
    mh64 = np.concatenate([mh1, mh1], axis=0)
    mh = np.concatenate([mh64, mh64], axis=0)

    sel = np.zeros((128, 3, 6), np.float32)
    sel[0:64, 0, 2] = 1.0
    sel[64:128, 0, 3] = 1.0
    sel[0:64, 1, 4] = 1.0
    sel[64:128, 1, 5] = 1.0
    sel[0:64, 2, 0] = 1.0
    sel[64:128, 2, 1] = 1.0

    tri = (np.arange(128)[None, :] >= np.arange(128)[:, None]).astype(bf)
    pscale = np.float32(1.0 / (T ** 0.5 + 1e-6))

    in_maps = []
    for c in range(NCORES):
        b, g = c // 4, c % 4
        in_maps.append({
            "xT": np.ascontiguousarray(x[b].T).astype(bf),
            "wq": np.ascontiguousarray(Wq[256 * g:256 * (g + 1)].T).astype(bf),
            "wk": np.ascontiguousarray(Wk[128 * g:128 * (g + 1)].T).astype(bf),
            "wv": np.ascontiguousarray(Wv[128 * g:128 * (g + 1)].T).astype(bf),
            "wp": np.ascontiguousarray((Wproj[:, 256 * g:256 * (g + 1)] * pscale).T).astype(bf),
            "gm": gm, "mh": mh, "sel": sel, "tri": tri,
        })
    return in_maps


def kernel(x, cos, sin, Wq, Wk, Wv, Wproj, w_braid):
    x = np.asarray(x, np.float32)
    nc = _get_program()
    in_maps = _host_inputs(np.asarray(x, np.float32), np.asarray(cos), np.asarray(sin),
                           np.asarray(Wq, np.float32), np.asarray(Wk, np.float32),
                           np.asarray(Wv, np.float32), np.asarray(Wproj, np.float32),
                           np.asarray(w_braid, np.float32))
    res = run_bass_kernel_spmd(nc, in_maps, list(range(NCORES)))
    out = np.zeros((2, T, C), np.float32)
    for c in range(NCORES):
        out[c // 4] += res.results[c]["outp"].astype(np.float32)
        out[c // 4] += res.results[c]["outp1"].astype(np.float32)
    return out


# revision 12
# speedup vs baseline: 1.4899x; 1.0211x over previous
"""Braid causal self-attention Trainium2 kernel (8-core SPMD).

Sharding: data-parallel over batch (2) x tensor-parallel over head groups (4).
Core c handles batch b=c//4, q-heads [4g:4g+4], kv-heads [2g:2g+2], g=c%4.
Each core computes a partial projection output (Wproj input-dim shard);
partials are summed on the host (bf16 partials, fp32 host sum).

Key structure (v3):
  - q/k are only needed through the braid scores s_q/s_k: with
    g[d,t] = braid/rotary-folded weights and mh[d,t] = sqrt(cos^2+sin^2),
    s = (sum_d q*g) * rsqrt(mean_d (q*mh)^2 + eps); rotary+rmsnorm are
    never materialized. The per-chunk score tail (Ln/Exp rsqrt, DRAM
    bounce) is pipelined with the projections.
  - attn = sigmoid(s_q[i] + s_k[j]): s_q rows are partition-broadcast by
    DMA from a DRAM bounce (no matmul, no psum), the per-key-block s_k
    column rides as the ACT bias, and the two q-heads sharing a kv head
    are stacked so one sigmoid call covers both. Causal masking is a
    128-wide triangular multiply on diagonal blocks only; attn@v uses
    partial-width matmuls so sub-diagonal strips are never touched.
  - All large matmuls (projections, attn@v, output projection) run in
    bf16; the braid score path stays fp32/f32r. v is transposed with the
    DMA transpose XBAR. Output projection is emitted in quarters as yt
    halves complete, overlapping the attention phase.
"""
import numpy as np
from contextlib import ExitStack

import ml_dtypes

import concourse.bass as bass
import concourse.mybir as mybir
import concourse.tile as tile
from concourse import bacc
from concourse.bass_utils import run_bass_kernel_spmd

F32 = mybir.dt.float32
F32R = mybir.dt.float32r
BF16 = mybir.dt.bfloat16
AF = mybir.ActivationFunctionType

T = 2048
C = 1024
D = 64
EPS = 1e-6
NCORES = 8


def build_program():
    nc = bacc.Bacc()
    dp = nc.declare_dram_parameter
    xT_d = dp("xT", [128, 4, 8, 512], BF16, isOutput=False)  # x[b].T pre-tiled
    wq_d = dp("wq", [128, 8, 256], BF16, isOutput=False)  # Wq[group].T pre-tiled
    wk_d = dp("wk", [128, 8, 128], BF16, isOutput=False)
    wv_d = dp("wv", [128, 8, 128], BF16, isOutput=False)
    wp_d = dp("wp", [128, 2, C], BF16, isOutput=False)    # Wproj[:, group].T pre-tiled (prescaled)
    gm_d = dp("gm", [128, T], F32, isOutput=False)        # braid g (2-head dup)
    mh_d = dp("mh", [128, T], F32, isOutput=False)        # sqrt(cos^2+sin^2) (2-head dup)
    sel_d = dp("sel", [128, 3, 6], F32, isOutput=False)   # head selector masks
    tri_d = dp("tri", [128, 2, 128], BF16, isOutput=False)  # tri(i>=j), 2-head dup
    out_d = dp("outp", [T, C], BF16, isOutput=True)
    out1_d = dp("outp1", [T, C], BF16, isOutput=True)

    with tile.TileContext(nc) as tc, \
         nc.allow_low_precision("bf16 matmuls fit the 2e-2 tolerance; score path stays fp32"), \
         ExitStack() as ctx:
        cons = ctx.enter_context(tc.tile_pool(name="cons", bufs=1))
        work = ctx.enter_context(tc.tile_pool(name="work", bufs=1))

        # ---- constants / weights in SBUF (DMAs spread across queues) ----
        wq_s = cons.tile([128, 8, 256], BF16)
        wk_s = cons.tile([128, 8, 128], BF16)
        wv_s = cons.tile([128, 8, 128], BF16)
        wp_s = cons.tile([128, 2, C], BF16)
        sel_s = cons.tile([128, 3, 6], F32R)
        tri_s = cons.tile([128, 2, 128], BF16)
        eps_t = cons.tile([128, 1], F32)
        gm_s = cons.tile([128, T], F32)
        mh_s = cons.tile([128, T], F32)
        xT_s = cons.tile([128, 4, 8, 512], BF16)
        nc.sync.dma_start(out=wq_s[:], in_=wq_d.ap())
        nc.sync.dma_start(out=xT_s[:, 0], in_=xT_d.ap()[:, 0])
        nc.gpsimd.dma_start(out=wk_s[:], in_=wk_d.ap())
        nc.gpsimd.dma_start(out=wv_s[:], in_=wv_d.ap())
        nc.gpsimd.dma_start(out=xT_s[:, 1], in_=xT_d.ap()[:, 1])
        nc.sync.dma_start(out=gm_s[:], in_=gm_d.ap())
        nc.gpsimd.dma_start(out=mh_s[:], in_=mh_d.ap())
        nc.sync.dma_start(out=xT_s[:, 2], in_=xT_d.ap()[:, 2])
        nc.gpsimd.dma_start(out=wp_s[:], in_=wp_d.ap())
        nc.sync.dma_start(out=sel_s[:], in_=sel_d.ap().bitcast(F32R))
        nc.sync.dma_start(out=tri_s[:], in_=tri_d.ap())
        nc.gpsimd.dma_start(out=xT_s[:, 3], in_=xT_d.ap()[:, 3])
        nc.vector.memset(eps_t[:], EPS)

        # long-lived work tiles
        vT = work.tile([128, T], BF16)
        v_td = work.tile([128, T], BF16)  # 16 blocks of [t128, oc128]
        scomp = work.tile([6, T], F32)
        stil = work.tile([6, T], F32)     # s-tilde accumulator (pre-rsqrt)
        kcolA = work.tile([128, 2, 8], F32)   # s_k columns: [j, kh, jb] jb 0-7
        kcolB = work.tile([128, 2, 8], F32)   # s_k columns jb 8-15
        r1 = work.tile([6, T], F32)
        psq_s = work.tile([6, T], F32)
        rq = work.tile([6, T], F32)
        yt0 = work.tile([128, T], BF16)  # heads 0,1 output (d-major)
        yt1 = work.tile([128, T], BF16)

        ksc0_d = nc.dram_tensor("kscratch0", [2, 1024], F32)
        ksc1_d = nc.dram_tensor("kscratch1", [2, 1024], F32)
        sq_d = nc.dram_tensor("sqscratch", [4, T], F32)

        # ==== phase 1: projections with fused braid reductions ====
        # cn-major: for each 512-column chunk, project all four row-tiles
        # (q0, q1, k, v), compute the braid products from PSUM, fold them
        # into the per-chunk selector matmuls, and finish the chunk's
        # scores (rsqrt + DRAM bounce) so attention can start early.
        with tc.tile_pool(name="bpool", bufs=2) as bp, \
             tc.tile_pool(name="pp1", bufs=2, space="PSUM") as pp1, \
             tc.tile_pool(name="pp2", bufs=2, space="PSUM") as pp2:
            tiles = [(wq_s, 0, 0), (wq_s, 128, 1), (wk_s, 0, 2), (wv_s, 0, 3)]
            for cn in range(4):
                sl = slice(512 * cn, 512 * cn + 512)
                pss_t = pp2.tile([6, 512], F32, tag="pss")
                psq_t = pp2.tile([6, 512], F32, tag="psq")
                for w_s, oc0, t_i in tiles:
                    ps = pp1.tile([128, 512], F32, tag="pj")
                    for kt in range(8):
                        nc.tensor.matmul(
                            ps[:], w_s[:, kt, oc0:oc0 + 128],
                            xT_s[:, cn, kt, :],
                            start=(kt == 0), stop=(kt == 7))
                    if t_i == 3:
                        nc.vector.tensor_copy(vT[:, sl], ps[:])
                    else:
                        a_t = bp.tile([128, 512], F32R, tag="a")
                        b_t = bp.tile([128, 512], F32, tag="b")
                        b2_t = bp.tile([128, 512], F32R, tag="b2")
                        nc.vector.tensor_mul(a_t[:], ps[:], gm_s[:, sl])
                        nc.vector.tensor_mul(b_t[:], ps[:], mh_s[:, sl])
                        nc.gpsimd.tensor_mul(b2_t[:], b_t[:], b_t[:])
                        nc.tensor.matmul(pss_t[:], sel_s[:, t_i, :], a_t[:],
                                         start=(t_i == 0), stop=(t_i == 2))
                        nc.tensor.matmul(psq_t[:], sel_s[:, t_i, :], b2_t[:],
                                         start=(t_i == 0), stop=(t_i == 2))
                # per-chunk score tail: s = stil * exp(-0.5*ln(ssq/64+eps)).
                # cn 0/1 run chunk-wise so attention can start early; cn 2/3
                # are merged into one Ln/Exp pair to bound sigmoid-set
                # table thrashing on the ACT engine.
                nc.vector.tensor_copy(stil[:, sl], pss_t[:])
                nc.vector.tensor_copy(psq_s[:, sl], psq_t[:])
                if cn < 2:
                    nc.scalar.activation(r1[:, sl], psq_s[:, sl], AF.Ln,
                                         bias=eps_t[0:6], scale=1.0 / 64.0)
                    nc.scalar.activation(rq[:, sl], r1[:, sl], AF.Exp, scale=-0.5)
                    nc.vector.tensor_mul(scomp[:, sl], stil[:, sl], rq[:, sl])
                    nc.sync.dma_start(out=ksc0_d.ap()[:, 512 * cn:512 * cn + 512],
                                      in_=scomp[0:2, sl])
                    nc.sync.dma_start(out=sq_d.ap()[:, sl], in_=scomp[2:6, sl])
                if cn == 1:
                    nc.gpsimd.dma_start(
                        out=kcolA[:],
                        in_=ksc0_d.ap().rearrange("r (b j) -> j r b", j=128))
                # v transpose for this chunk via the DMA transpose XBAR
                for k in range(4):
                    jb = 4 * cn + k
                    nc.sync.dma_start(out=v_td[:, 128 * jb:128 * jb + 128],
                                 in_=vT[:, 128 * jb:128 * jb + 128],
                                 transpose=True)

        # ================= phase 3: attention + streamed projection ======
        with tc.tile_pool(name="sqpool", bufs=3) as sqp, \
             tc.tile_pool(name="atpool", bufs=8) as atp, \
             tc.tile_pool(name="ostage", bufs=4) as osp, \
             tc.tile_pool(name="pp3y", bufs=3, space="PSUM") as pp3y, \
             tc.tile_pool(name="pp4", bufs=2, space="PSUM") as pp4:
            for hs in (0, 1024):
                if hs == 1024:
                    # deferred score tail for chunks 2-3: lands in the ACT
                    # queue after the hs=0 sigmoids, so the table switch
                    # happens exactly once between the halves
                    h2 = slice(1024, 2048)
                    nc.scalar.activation(r1[:, h2], psq_s[:, h2], AF.Ln,
                                         bias=eps_t[0:6], scale=1.0 / 64.0)
                    nc.scalar.activation(rq[:, h2], r1[:, h2], AF.Exp, scale=-0.5)
                    nc.vector.tensor_mul(scomp[:, h2], stil[:, h2], rq[:, h2])
                    nc.sync.dma_start(out=ksc1_d.ap()[:], in_=scomp[0:2, h2])
                    nc.sync.dma_start(out=sq_d.ap()[:, h2], in_=scomp[2:6, h2])
                    nc.gpsimd.dma_start(
                        out=kcolB[:],
                        in_=ksc1_d.ap().rearrange("r (b j) -> j r b", j=128))
                for kh in range(2):
                    h0 = 2 * kh
                    # s_q rows for both heads, partition-broadcast by DMA
                    sqb = sqp.tile([128, 2, 1024], F32, tag="sqb")
                    for hh in range(2):
                        qd = [nc.sync, nc.gpsimd][hh]
                        qd.dma_start(
                            out=sqb[:, hh, :],
                            in_=sq_d.ap()[h0 + hh:h0 + hh + 1, hs:hs + 1024]
                                .to_broadcast((128, 1024)))
                    jmax = (hs + 1024) // 128
                    y_ps = pp3y.tile([128, 1024], F32, tag="yps")
                    # last jb touching each 512-wide psum window
                    last_w = [min((hs + 512 * ck + 512) // 128, jmax) - 1
                              for ck in range(2)]
                    for jb in range(jmax):
                        vstart = max(hs, 128 * jb)
                        voff = vstart - hs   # first live col within the window
                        at_t = atp.tile([128, 2, 1024], BF16, tag="att")
                        # attn = sigmoid(s_q[i] + s_k[j]) for both heads in
                        # one call; s_k column as ACT bias.
                        nc.scalar.activation(
                            at_t[:, :, voff:1024],
                            sqb[:, :, voff:1024],
                            AF.Sigmoid,
                            bias=(kcolA if jb < 8 else kcolB)[:, kh, jb % 8:jb % 8 + 1])
                        # causal tri mask on the diagonal block only
                        if 128 * jb >= hs:
                            nc.vector.tensor_mul(
                                at_t[:, :, voff:voff + 128],
                                at_t[:, :, voff:voff + 128],
                                tri_s[:])
                        # attn @ v accumulation: partial-width matmuls start
                        # at the causal boundary; head hh lands on psum
                        # partitions [64*hh, 64*hh+64)
                        for ck in range(voff // 512, 2):
                            lo = max(voff, 512 * ck)
                            for hh in range(2):
                                nc.tensor.matmul(
                                    y_ps[64 * hh:64 * hh + 64, lo:512 * ck + 512],
                                    v_td[:, 128 * jb + 64 * kh:128 * jb + 64 * kh + 64],
                                    at_t[:, hh, lo:512 * ck + 512],
                                    start=(jb == 0),
                                    stop=(last_w[ck] == jb))
                    yt_dst = yt0 if kh == 0 else yt1
                    nc.vector.tensor_copy(yt_dst[:, hs:hs + 1024], y_ps[:])

                    # stream out the finished quarter of the output
                    # projection (yt[kh][:, hs:hs+1024] is now complete)
                    od = out_d if kh == 0 else out1_d
                    yt_src = yt_dst
                    tail = hs == 1024 and kh == 1
                    for ti in range(8):
                        tt = hs // 128 + ti
                        o_t = osp.tile([128, C], BF16, tag="ost")
                        for cn in range(2):
                            ps_o = pp4.tile([128, 512], F32, tag="opj")
                            nc.tensor.matmul(ps_o[:],
                                             yt_src[:, 128 * tt:128 * tt + 128],
                                             wp_s[:, kh, 512 * cn:512 * cn + 512],
                                             start=True, stop=True)
                            if tail and cn == 1:
                                nc.scalar.copy(o_t[:, 512 * cn:512 * cn + 512], ps_o[:])
                            else:
                                nc.vector.tensor_copy(o_t[:, 512 * cn:512 * cn + 512], ps_o[:])
                        qd = [nc.sync, nc.gpsimd][ti % 2]
                        qd.dma_start(
                            out=od.ap()[128 * tt:128 * tt + 128, :],
                            in_=o_t[:])

    nc.compile()
    return nc


_PROGRAM = None


def _get_program():
    global _PROGRAM
    if _PROGRAM is None:
        _PROGRAM = build_program()
    return _PROGRAM


def _host_inputs(x, cos, sin, Wq, Wk, Wv, Wproj, w_braid):
    bf = ml_dtypes.bfloat16
    cos2 = cos[:, 0, :].astype(np.float32)   # [T, 32]
    sin2 = sin[:, 0, :].astype(np.float32)
    wb = w_braid.astype(np.float32)
    g64 = np.empty((64, T), np.float32)
    g64[:32] = wb[:32, None] * cos2.T - wb[32:, None] * sin2.T
    g64[32:] = wb[32:, None] * cos2.T + wb[:32, None] * sin2.T
    gm = np.concatenate([g64, g64], axis=0)
    mh1 = np.sqrt(cos2.T ** 2 + sin2.T ** 2).astype(np.float32)  # [32, T]
    mh64 = np.concatenate([mh1, mh1], axis=0)
    mh = np.concatenate([mh64, mh64], axis=0)

    sel = np.zeros((128, 3, 6), np.float32)
    sel[0:64, 0, 2] = 1.0
    sel[64:128, 0, 3] = 1.0
    sel[0:64, 1, 4] = 1.0
    sel[64:128, 1, 5] = 1.0
    sel[0:64, 2, 0] = 1.0
    sel[64:128, 2, 1] = 1.0

    tri = (np.arange(128)[None, :] >= np.arange(128)[:, None]).astype(bf)
    pscale = np.float32(1.0 / (T ** 0.5 + 1e-6))

    in_maps = []
    for c in range(NCORES):
        b, g = c // 4, c % 4
        in_maps.append({
            "xT": np.ascontiguousarray(
                x[b].T.reshape(8, 128, 4, 512).transpose(1, 2, 0, 3)).astype(bf),
            "wq": np.ascontiguousarray(
                Wq[256 * g:256 * (g + 1)].T.reshape(8, 128, 256).transpose(1, 0, 2)).astype(bf),
            "wk": np.ascontiguousarray(
                Wk[128 * g:128 * (g + 1)].T.reshape(8, 128, 128).transpose(1, 0, 2)).astype(bf),
            "wv": np.ascontiguousarray(
                Wv[128 * g:128 * (g + 1)].T.reshape(8, 128, 128).transpose(1, 0, 2)).astype(bf),
            "wp": np.ascontiguousarray(
                (Wproj[:, 256 * g:256 * (g + 1)] * pscale).T
                .reshape(2, 128, 1024).transpose(1, 0, 2)).astype(bf),
            "gm": gm, "mh": mh, "sel": sel,
            "tri": np.ascontiguousarray(np.stack([tri, tri], axis=1)),
        })
    return in_maps


def kernel(x, cos, sin, Wq, Wk, Wv, Wproj, w_braid):
    x = np.asarray(x, np.float32)
    nc = _get_program()
    in_maps = _host_inputs(np.asarray(x, np.float32), np.asarray(cos), np.asarray(sin),
                           np.asarray(Wq, np.float32), np.asarray(Wk, np.float32),
                           np.asarray(Wv, np.float32), np.asarray(Wproj, np.float32),
                           np.asarray(w_braid, np.float32))
    res = run_bass_kernel_spmd(nc, in_maps, list(range(NCORES)))
    out = np.zeros((2, T, C), np.float32)
    for c in range(NCORES):
        out[c // 4] += res.results[c]["outp"].astype(np.float32)
        out[c // 4] += res.results[c]["outp1"].astype(np.float32)
    return out
